# revision 8
# baseline (speedup 1.0000x reference)
"""Trainium2 Bass kernel for nn_DA_MoCoNNQQ_Disc_TCN_Siam (retrieval_knn).

Strategy:
- Data-parallel over batch B=256 across 8 cores (32 rows each) for the four
  TCN encoder passes. Only the last 125 timesteps of each sequence matter
  (receptive field of the dilated TCN stack), so the host pre-slices and
  pre-transposes inputs to channel-major [64, 32, 128] segments.
- Encoder matmuls: fp32 for the q_s / k_t passes (their embeddings feed the
  cdist/argmin whose top-2 gaps are ~2e-4 — needs full fp32), float32r
  (tf32-rate) for q_t / k_s (their consumers have loose tolerance).
- Tiny [64, 32] embeddings are AllGathered across the 8 cores; every core
  then computes the full-batch tail (MLPs, cdist/argmax, block logits) and
  1/8 of the big queue logits columns.
- Host reassembles the full outputs.
"""
import numpy as np

import concourse.bass as bass
import concourse.tile as tile
from concourse import mybir, bacc
from concourse.bass_utils import run_bass_kernel_spmd

F32 = mybir.dt.float32
F32R = mybir.dt.float32r
I32 = mybir.dt.int32
U32 = mybir.dt.uint32
AF = mybir.ActivationFunctionType
OP = mybir.AluOpType

N_CORES = 8
B = 256
BL = B // N_CORES          # 32 batch rows per core
L = 1024
C = 64
NUM_LAYERS = 5
KQ = 24576
KQL = KQ // N_CORES        # 3072 queue columns per core
HID = 256
NST = 32
W = 125                    # receptive field of the TCN at the last step
SEGW = 128                 # padded per-batch segment width
T_INV = 1.0 / 0.07
EPS = 1e-12

# per-layer needed input ranges: R[l] timesteps of x_l feed the final output
R = [125, 121, 113, 97, 65, 1]
# gather order of the four passes
PASS_QS, PASS_QT, PASS_KS, PASS_KT = 0, 1, 2, 3
# (pass, precision): q_s & k_t exact
PASS_DT = {PASS_QS: F32, PASS_KT: F32, PASS_QT: F32R, PASS_KS: F32R}


def _conv_groups(c):
    """Split 32 segments into groups with group_size*c <= 512."""
    g = min(32, max(1, 512 // c))
    out = []
    s = 0
    while s < 32:
        out.append((s, min(g, 32 - s)))
        s += g
    return out


def build_nc():
    nc = bacc.Bacc("TRN2", target_bir_lowering=False, debug=False,
                   num_devices=N_CORES)

    # ---------------- DRAM parameters ----------------
    seq_in = {}
    for p in range(4):
        seq_in[p] = nc.dram_tensor(f"seq{p}", [C, BL, SEGW], PASS_DT[p],
                                   kind="ExternalInput")
    wpk = {F32: nc.dram_tensor("wpk_f", [30, C, C], F32, kind="ExternalInput"),
           F32R: nc.dram_tensor("wpk_r", [30, C, C], F32R, kind="ExternalInput")}
    bias_d = nc.dram_tensor("biasd", [10, C], F32, kind="ExternalInput")
    pjw1_d = nc.dram_tensor("pjw1", [C, HID], F32, kind="ExternalInput")
    pjw2_d = nc.dram_tensor("pjw2p", [128, 2, C], F32, kind="ExternalInput")
    pjb1_d = nc.dram_tensor("pjb1p", [128, 2], F32, kind="ExternalInput")
    pjb2_d = nc.dram_tensor("pjb2p", [C, 1], F32, kind="ExternalInput")
    prw1_d = nc.dram_tensor("prw1", [C + NST, HID], F32, kind="ExternalInput")
    prw2_d = nc.dram_tensor("prw2p", [128, 2, 2], F32, kind="ExternalInput")
    prb1_d = nc.dram_tensor("prb1p", [128, 2], F32, kind="ExternalInput")
    prb2_d = nc.dram_tensor("prb2p", [2, 1], F32, kind="ExternalInput")
    dw1_d = nc.dram_tensor("dw1", [C, HID], F32, kind="ExternalInput")
    dw2_d = nc.dram_tensor("dw2p", [128, 2, 1], F32, kind="ExternalInput")
    db1_d = nc.dram_tensor("db1p", [128, 2], F32, kind="ExternalInput")
    db2_d = nc.dram_tensor("db2p", [1, 1], F32, kind="ExternalInput")
    statT_d = nc.dram_tensor("statT", [NST, B], F32, kind="ExternalInput")
    qs_d = nc.dram_tensor("queue_s", [C, KQL], F32R, kind="ExternalInput")
    qt_d = nc.dram_tensor("queue_t", [C, KQL], F32R, kind="ExternalInput")

    out_qs = {"s": nc.dram_tensor("oq_s", [B, KQL], F32, kind="ExternalOutput"),
              "t": nc.dram_tensor("oq_t", [B, KQL], F32, kind="ExternalOutput")}
    out_blk = {"s": nc.dram_tensor("ob_s", [B, B], F32, kind="ExternalOutput"),
               "t": nc.dram_tensor("ob_t", [B, B], F32, kind="ExternalOutput")}
    out_ts = nc.dram_tensor("o_ts", [B, B], F32, kind="ExternalOutput")
    out_lab = nc.dram_tensor("o_lab", [B], I32, kind="ExternalOutput")
    out_pd = nc.dram_tensor("o_pd", [2 * B], F32, kind="ExternalOutput")
    out_ys = nc.dram_tensor("o_ys", [2, B], F32, kind="ExternalOutput")

    with tile.TileContext(nc) as tc:
        with (
            tc.tile_pool(name="const", bufs=1) as constp,
            tc.tile_pool(name="act", bufs=1) as actp,
            tc.tile_pool(name="small", bufs=1) as smallp,
            tc.tile_pool(name="work", bufs=2) as workp,
            tc.tile_pool(name="ps", bufs=8, space="PSUM") as psp,
            tc.tile_pool(name="dram", bufs=1, space="DRAM") as dramp,
        ):
            # ---------------- constants / weights ----------------
            w_sb = {}
            for dt in (F32, F32R):
                t = constp.tile([C, 30, C], dt, tag=f"w_{dt}")
                nc.sync.dma_start(out=t[:], in_=wpk[dt][:].rearrange("i p o -> p i o"))
                w_sb[dt] = t
            bias_sb = constp.tile([C, 10], F32, tag="bias")
            nc.sync.dma_start(out=bias_sb[:], in_=bias_d[:].rearrange("i c -> c i"))
            ones64 = constp.tile([C, 1], F32, tag="ones64")
            nc.vector.memset(ones64[:], 1.0)
            ones1x64 = constp.tile([1, C], F32, tag="ones1x64")
            nc.vector.memset(ones1x64[:], 1.0)
            ones1x128 = constp.tile([1, 128], F32, tag="ones1x128")
            nc.vector.memset(ones1x128[:], 1.0)

            pjw1 = constp.tile([C, HID], F32, tag="pjw1")
            nc.sync.dma_start(out=pjw1[:], in_=pjw1_d[:])
            pjw2 = constp.tile([128, 2, C], F32, tag="pjw2")
            nc.sync.dma_start(out=pjw2[:], in_=pjw2_d[:])
            pjb1 = constp.tile([128, 2], F32, tag="pjb1")
            nc.sync.dma_start(out=pjb1[:], in_=pjb1_d[:])
            pjb2 = constp.tile([C, 1], F32, tag="pjb2")
            nc.sync.dma_start(out=pjb2[:], in_=pjb2_d[:])
            prw1 = constp.tile([C + NST, HID], F32, tag="prw1")
            nc.sync.dma_start(out=prw1[:], in_=prw1_d[:])
            prw2 = constp.tile([128, 2, 2], F32, tag="prw2")
            nc.sync.dma_start(out=prw2[:], in_=prw2_d[:])
            prb1 = constp.tile([128, 2], F32, tag="prb1")
            nc.sync.dma_start(out=prb1[:], in_=prb1_d[:])
            prb2 = constp.tile([2, 1], F32, tag="prb2")
            nc.sync.dma_start(out=prb2[:], in_=prb2_d[:])
            dw1 = constp.tile([C, HID], F32, tag="dw1")
            nc.sync.dma_start(out=dw1[:], in_=dw1_d[:])
            dw2 = constp.tile([128, 2, 1], F32, tag="dw2")
            nc.sync.dma_start(out=dw2[:], in_=dw2_d[:])
            db1 = constp.tile([128, 2], F32, tag="db1")
            nc.sync.dma_start(out=db1[:], in_=db1_d[:])
            db2 = constp.tile([1, 1], F32, tag="db2")
            nc.sync.dma_start(out=db2[:], in_=db2_d[:])

            queue_sb = {}
            for tag, d in (("s", qs_d), ("t", qt_d)):
                t = constp.tile([C, KQL], F32R, tag=f"queue_{tag}")
                nc.sync.dma_start(out=t[:], in_=d[:])
                queue_sb[tag] = t

            # ---------------- TCN encoder: 4 passes ----------------
            x5c = {}
            for p in range(4):
                dt = PASS_DT[p]
                x = actp.tile([C, BL, SEGW], dt, tag="x0")
                nc.sync.dma_start(out=x[:], in_=seq_in[p][:])
                for lyr in range(NUM_LAYERS):
                    d = 2 ** lyr
                    c2 = R[lyr + 1] + (R[lyr + 1] % 2)   # fp32r needs even N
                    c1 = c2 + 2 * d
                    wbase = lyr * 6
                    h1 = actp.tile([C, BL, SEGW], dt, tag=f"h1{lyr % 2}")
                    # conv1 + bias + relu
                    o1 = SEGW - c1
                    for (s0, g) in _conv_groups(c1):
                        pm = psp.tile([C, g, c1], F32, tag="pp")
                        pmv = pm[:]
                        for k in range(3):
                            off = o1 - (2 - k) * d
                            nc.tensor.matmul(
                                pmv, w_sb[dt][:, wbase + k, :],
                                x[:, s0:s0 + g, off:off + c1],
                                start=(k == 0), stop=(k == 2))
                        nc.scalar.activation(
                            h1[:, s0:s0 + g, o1:SEGW], pmv, AF.Relu,
                            bias=bias_sb[:, 2 * lyr:2 * lyr + 1])
                    # conv2 + bias + relu, then residual (+relu on layer 0)
                    o2 = SEGW - c2
                    xn = actp.tile([C, BL, SEGW], dt, tag=f"x{(lyr + 1) % 3}")
                    for (s0, g) in _conv_groups(c2):
                        pm = psp.tile([C, g, c2], F32, tag="pp")
                        pmv = pm[:]
                        for k in range(3):
                            off = o2 - (2 - k) * d
                            nc.tensor.matmul(
                                pmv, w_sb[dt][:, wbase + 3 + k, :],
                                h1[:, s0:s0 + g, off:off + c2],
                                start=(k == 0), stop=(k == 2))
                        h2g = workp.tile([C, g, c2], F32, tag="h2g")
                        nc.scalar.activation(
                            h2g[:], pmv, AF.Relu,
                            bias=bias_sb[:, 2 * lyr + 1:2 * lyr + 2])
                        if lyr == 0:
                            tmp = workp.tile([C, g, c2], F32, tag="res0")
                            nc.vector.tensor_tensor(
                                tmp[:], h2g[:], x[:, s0:s0 + g, o2:SEGW], OP.add)
                            nc.vector.tensor_scalar_max(
                                xn[:, s0:s0 + g, o2:SEGW], tmp[:], 0.0)
                        else:
                            nc.vector.tensor_tensor(
                                xn[:, s0:s0 + g, o2:SEGW], h2g[:],
                                x[:, s0:s0 + g, o2:SEGW], OP.add)
                    x = xn
                # copy out the last-step features [64, 32]
                xc = smallp.tile([C, BL], F32, tag=f"x5c_{p}")
                nc.vector.tensor_copy(xc[:], x[:, :, SEGW - 1:SEGW])
                x5c[p] = xc

            # ---------------- normalize the four embeddings ----------------
            nrm_ps = psp.tile([1, 512], F32, tag="pp")
            sq = {}
            for p in range(4):
                s = workp.tile([C, BL], F32, tag="sq")
                nc.vector.tensor_tensor(s[:], x5c[p][:], x5c[p][:], OP.mult)
                nc.tensor.matmul(nrm_ps[0:1, 32 * p:32 * p + 32], ones64[:], s[:],
                                 start=True, stop=True)
                sq[p] = s
            nrm = smallp.tile([1, 128], F32, tag="nrm")
            nc.scalar.sqrt(nrm[:], nrm_ps[0:1, 0:128])
            nc.vector.tensor_scalar_max(nrm[:], nrm[:], EPS)
            rec = smallp.tile([1, 128], F32, tag="rec")
            nc.vector.reciprocal(rec[:], nrm[:])
            bc_ps = psp.tile([C, 128], F32, tag="pp")
            nc.tensor.matmul(bc_ps[:], ones1x64[:], rec[:], start=True, stop=True)
            emb = {}
            for p in range(4):
                e = smallp.tile([C, BL], F32, tag=f"emb_{p}")
                nc.vector.tensor_tensor(e[:], x5c[p][:], bc_ps[:, 32 * p:32 * p + 32],
                                        OP.mult)
                emb[p] = e

            # ---------------- all-gather the embeddings ----------------
            b_in = dramp.tile([4 * C, BL], F32)
            b_out = dramp.tile([N_CORES * 4 * C, BL], F32)
            for p in range(4):
                nc.sync.dma_start(out=b_in[C * p:C * (p + 1), :], in_=emb[p][:])
            nc.gpsimd.collective_compute(
                "AllGather", OP.bypass,
                replica_groups=[list(range(N_CORES))],
                ins=[b_in.opt()], outs=[b_out.opt()])
            emb_all = smallp.tile([C, 4, B], F32, tag="emb_all")
            src = b_out[:].rearrange("(k p c) b -> p c k b", k=N_CORES, p=4)
            for p in range(4):
                nc.sync.dma_start(
                    out=emb_all[:, p, :].rearrange("c (k b) -> c k b", k=N_CORES),
                    in_=src[p])

            q_s_all = emb_all[:, PASS_QS, :]
            q_t_all = emb_all[:, PASS_QT, :]
            k_s_all = emb_all[:, PASS_KS, :]
            k_t_all = emb_all[:, PASS_KT, :]

            # ---------------- projection MLP p_q = l2n(mlp(q)) * (1/T) -------
            pq_scaled = {}
            pqn_ps = psp.tile([1, 512], F32, tag="pp")
            praw = {}
            for i, (tag, qv) in enumerate((("s", q_s_all), ("t", q_t_all))):
                hts = []
                for m in range(2):
                    hp = psp.tile([128, 256], F32, tag="pp")
                    nc.tensor.matmul(hp[:], pjw1[:, 128 * m:128 * (m + 1)], qv,
                                     start=True, stop=True)
                    ht = workp.tile([128, B], F32, tag=f"pqh{m}")
                    nc.scalar.activation(ht[:], hp[:], AF.Relu,
                                         bias=pjb1[:, m:m + 1])
                    hts.append(ht)
                op = psp.tile([C, 256], F32, tag="pp")
                for s in range(2):
                    nc.tensor.matmul(op[:], pjw2[:, s, :], hts[s][:],
                                     start=(s == 0), stop=(s == 1))
                pr = smallp.tile([C, B], F32, tag=f"praw_{tag}")
                nc.scalar.activation(pr[:], op[:], AF.Identity, bias=pjb2[:])
                praw[tag] = pr
                s2 = workp.tile([C, B], F32, tag="sq2")
                nc.vector.tensor_tensor(s2[:], pr[:], pr[:], OP.mult)
                nc.tensor.matmul(pqn_ps[0:1, 256 * i:256 * (i + 1)], ones64[:],
                                 s2[:], start=True, stop=True)
            pqn = smallp.tile([1, 512], F32, tag="pqn")
            nc.scalar.sqrt(pqn[:], pqn_ps[:])
            nc.vector.tensor_scalar_max(pqn[:], pqn[:], EPS)
            pqr = smallp.tile([1, 512], F32, tag="pqr")
            nc.vector.reciprocal(pqr[:], pqn[:])
            nc.vector.tensor_scalar_mul(pqr[:], pqr[:], T_INV)
            for i, tag in enumerate(("s", "t")):
                bp = psp.tile([C, 512], F32, tag="pp")
                nc.tensor.matmul(bp[0:C, 0:256], ones1x64[:],
                                 pqr[0:1, 256 * i:256 * (i + 1)],
                                 start=True, stop=True)
                pq = smallp.tile([C, B], F32, tag=f"pq_{tag}")
                nc.vector.tensor_tensor(pq[:], praw[tag][:], bp[0:C, 0:256],
                                        OP.mult)
                pq_scaled[tag] = pq

            # ---------------- queue logits (f32r, col-sharded) ----------------
            for tag in ("s", "t"):
                pq_r = smallp.tile([C, B], F32R, tag=f"pqr_{tag}")
                nc.vector.tensor_copy(pq_r[:], pq_scaled[tag][:])
                for m in range(2):
                    for n in range(KQL // 512):
                        qp = psp.tile([128, 512], F32, tag="pp")
                        nc.tensor.matmul(
                            qp[:], pq_r[:, 128 * m:128 * (m + 1)],
                            queue_sb[tag][:, 512 * n:512 * (n + 1)],
                            start=True, stop=True)
                        qs = workp.tile([128, 512], F32, tag="qout")
                        nc.vector.tensor_copy(qs[:], qp[:])
                        nc.sync.dma_start(
                            out=out_qs[tag][128 * m:128 * (m + 1),
                                            512 * n:512 * (n + 1)],
                            in_=qs[:])

            # ---------------- block logits + logits_ts (fp32) ----------------
            qt_scaled = smallp.tile([C, B], F32, tag="qt_scaled")
            nc.scalar.mul(qt_scaled[:], q_t_all, T_INV)
            blocks = [("s", pq_scaled["s"][:], k_s_all, out_blk["s"]),
                      ("t", pq_scaled["t"][:], k_t_all, out_blk["t"]),
                      ("ts", qt_scaled[:], q_s_all, out_ts)]
            for tag, lhs, rhs, od in blocks:
                for m in range(2):
                    bp2 = psp.tile([128, 256], F32, tag="pp")
                    nc.tensor.matmul(bp2[:], lhs[:, 128 * m:128 * (m + 1)], rhs,
                                     start=True, stop=True)
                    bs = workp.tile([128, B], F32, tag="bout")
                    nc.vector.tensor_copy(bs[:], bp2[:])
                    nc.sync.dma_start(out=od[128 * m:128 * (m + 1), :], in_=bs[:])

            # ---------------- cdist + argmin (exact fp32) ----------------
            sqq = workp.tile([C, B], F32, tag="sqq")
            nc.vector.tensor_tensor(sqq[:], q_s_all, q_s_all, OP.mult)
            nq_ps = psp.tile([1, 256], F32, tag="pp")
            nc.tensor.matmul(nq_ps[:], ones64[:], sqq[:], start=True, stop=True)
            nqs = smallp.tile([1, B], F32, tag="nqs")
            nc.scalar.copy(nqs[:], nq_ps[:])
            b2_ps = psp.tile([128, 256], F32, tag="pp")
            nc.tensor.matmul(b2_ps[:], ones1x128[:], nqs[:], start=True, stop=True)
            b2_sb = workp.tile([128, B], F32, tag="b2_sb")
            nc.vector.tensor_copy(b2_sb[:], b2_ps[:])
            for m in range(2):
                s_ps = psp.tile([128, 256], F32, tag="pp")
                nc.tensor.matmul(s_ps[:], k_t_all[:, 128 * m:128 * (m + 1)],
                                 q_s_all, start=True, stop=True)
                a_sb = workp.tile([128, B], F32, tag="a_sb")
                nc.vector.scalar_tensor_tensor(a_sb[:], s_ps[:], 2.0, b2_sb[:],
                                               OP.mult, OP.subtract)
                m8 = workp.tile([128, 8], F32, tag="m8")
                nc.vector.max(m8[:], a_sb[:])
                i8 = workp.tile([128, 8], U32, tag="i8")
                nc.vector.max_index(i8[:], m8[:], a_sb[:])
                li = workp.tile([128, 1], I32, tag="li")
                nc.vector.tensor_copy(li[:], i8[:, 0:1])
                nc.sync.dma_start(out=out_lab[128 * m:128 * (m + 1)], in_=li[:])

            # ---------------- pred_domain (fp32) ----------------
            q_rev = emb_all[:, 0:2, :]   # [64, 2, 256] == [q_s | q_t]
            hds = []
            for m in range(2):
                hp = psp.tile([128, 512], F32, tag="pp")
                nc.tensor.matmul(hp[:], dw1[:, 128 * m:128 * (m + 1)], q_rev,
                                 start=True, stop=True)
                hd = workp.tile([128, 2 * B], F32, tag=f"hd{m}")
                nc.scalar.activation(hd[:], hp[:], AF.Relu, bias=db1[:, m:m + 1])
                hds.append(hd)
            pd_ps = psp.tile([1, 512], F32, tag="pp")
            for s in range(2):
                nc.tensor.matmul(pd_ps[:], dw2[:, s, :], hds[s][:],
                                 start=(s == 0), stop=(s == 1))
            pd_sb = workp.tile([1, 2 * B], F32, tag="pd_sb")
            nc.scalar.activation(pd_sb[:], pd_ps[:], AF.Identity, bias=db2[:])
            nc.sync.dma_start(out=out_pd[:], in_=pd_sb[:])

            # ---------------- y_s head (fp32) ----------------
            rhs96 = workp.tile([C + NST, B], F32, tag="rhs96")
            nc.vector.tensor_copy(rhs96[0:C, :], q_s_all)
            nc.sync.dma_start(out=rhs96[C:C + NST, :], in_=statT_d[:])
            hys = []
            for m in range(2):
                hp = psp.tile([128, 256], F32, tag="pp")
                nc.tensor.matmul(hp[:], prw1[:, 128 * m:128 * (m + 1)], rhs96[:],
                                 start=True, stop=True)
                hy = workp.tile([128, B], F32, tag=f"hy{m}")
                nc.scalar.activation(hy[:], hp[:], AF.Relu, bias=prb1[:, m:m + 1])
                hys.append(hy)
            ys_ps = psp.tile([2, 256], F32, tag="pp")
            for s in range(2):
                nc.tensor.matmul(ys_ps[:], prw2[:, s, :], hys[s][:],
                                 start=(s == 0), stop=(s == 1))
            ys_sb = workp.tile([2, B], F32, tag="ys_sb")
            nc.scalar.activation(ys_sb[:], ys_ps[:], AF.Identity, bias=prb2[:])
            nc.sync.dma_start(out=out_ys[:], in_=ys_sb[:])

    nc.compile()
    return nc


_NC = None


def _get_nc():
    global _NC
    if _NC is None:
        _NC = build_nc()
    return _NC


def make_in_maps(inputs):
    """Host-side prepacking: slice/transpose/shard the full inputs."""
    f = np.float32
    seq_names = {PASS_QS: "sequence_q_s", PASS_QT: "sequence_q_t",
                 PASS_KS: "sequence_k_s", PASS_KT: "sequence_k_t"}
    c1w, c1b = np.asarray(inputs["c1w"], f), np.asarray(inputs["c1b"], f)
    c2w, c2b = np.asarray(inputs["c2w"], f), np.asarray(inputs["c2b"], f)
    wpk = np.zeros((30, C, C), f)
    biasd = np.zeros((10, C), f)
    for lyr in range(NUM_LAYERS):
        for k in range(3):
            wpk[lyr * 6 + k] = c1w[lyr, :, :, k].T
            wpk[lyr * 6 + 3 + k] = c2w[lyr, :, :, k].T
        biasd[2 * lyr] = c1b[lyr]
        biasd[2 * lyr + 1] = c2b[lyr]

    pjw2p = np.asarray(inputs["pjw2"], f).reshape(2, 128, C).transpose(1, 0, 2)
    pjb1p = np.asarray(inputs["pjb1"], f).reshape(2, 128).T
    prw2p = np.asarray(inputs["prw2"], f).reshape(2, 128, 2).transpose(1, 0, 2)
    prb1p = np.asarray(inputs["prb1"], f).reshape(2, 128).T
    dw2p = np.asarray(inputs["dw2"], f).reshape(2, 128, 1).transpose(1, 0, 2)
    db1p = np.asarray(inputs["db1"], f).reshape(2, 128).T

    shared = {
        "wpk_f": np.ascontiguousarray(wpk),
        "wpk_r": np.ascontiguousarray(wpk),
        "biasd": biasd,
        "pjw1": np.ascontiguousarray(np.asarray(inputs["pjw1"], f)),
        "pjw2p": np.ascontiguousarray(pjw2p),
        "pjb1p": np.ascontiguousarray(pjb1p),
        "pjb2p": np.asarray(inputs["pjb2"], f).reshape(C, 1),
        "prw1": np.ascontiguousarray(np.asarray(inputs["prw1"], f)),
        "prw2p": np.ascontiguousarray(prw2p),
        "prb1p": np.ascontiguousarray(prb1p),
        "prb2p": np.asarray(inputs["prb2"], f).reshape(2, 1),
        "dw1": np.ascontiguousarray(np.asarray(inputs["dw1"], f)),
        "dw2p": np.ascontiguousarray(dw2p),
        "db1p": np.ascontiguousarray(db1p),
        "db2p": np.asarray(inputs["db2"], f).reshape(1, 1),
        "statT": np.ascontiguousarray(np.asarray(inputs["static_s"], f).T),
    }
    in_maps = []
    for c in range(N_CORES):
        m = dict(shared)
        for p in range(4):
            seq = np.asarray(inputs[seq_names[p]], f)
            arr = np.zeros((C, BL, SEGW), f)
            arr[:, :, SEGW - W:] = seq[c * BL:(c + 1) * BL, L - W:, :] \
                .transpose(2, 0, 1)
            m[f"seq{p}"] = arr
        m["queue_s"] = np.ascontiguousarray(
            np.asarray(inputs["queue_s"], f)[:, c * KQL:(c + 1) * KQL])
        m["queue_t"] = np.ascontiguousarray(
            np.asarray(inputs["queue_t"], f)[:, c * KQL:(c + 1) * KQL])
        in_maps.append(m)
    return in_maps


def assemble(results):
    """Host-side reassembly of full outputs from per-core results."""
    f = np.float32
    r0 = results[0]
    logits_s = np.concatenate([r0["ob_s"]] + [results[c]["oq_s"]
                                              for c in range(N_CORES)], axis=1)
    logits_t = np.concatenate([r0["ob_t"]] + [results[c]["oq_t"]
                                              for c in range(N_CORES)], axis=1)
    logits_ts = r0["o_ts"]
    labels_ts = r0["o_lab"].astype(np.int32)
    pred_domain = r0["o_pd"].reshape(2 * B, 1)
    y_s = np.ascontiguousarray(r0["o_ys"].T)
    labels_s = np.arange(B, dtype=np.int32)
    labels_t = np.arange(B, dtype=np.int32)
    labels_domain = np.concatenate([np.ones((B, 1), f), np.zeros((B, 1), f)], 0)
    return (logits_s, labels_s, logits_t, labels_t, logits_ts, labels_ts,
            pred_domain, labels_domain, y_s)


def kernel(**inputs):
    nc = _get_nc()
    in_maps = make_in_maps(inputs)
    last_err = None
    for _ in range(3):
        try:
            res = run_bass_kernel_spmd(nc, in_maps, list(range(N_CORES)))
            return assemble(res.results)
        except Exception as e:  # compile/exec flakes: retry
            last_err = e
    raise last_err


# revision 10
# speedup vs baseline: 1.1604x; 1.1604x over previous
"""Trainium2 Bass kernel for nn_DA_MoCoNNQQ_Disc_TCN_Siam (retrieval_knn).

Strategy:
- Data-parallel over batch B=256 across 8 cores (32 rows each) for the four
  TCN encoder passes. Only the last 125 timesteps of each sequence matter
  (receptive field of the dilated TCN stack), so the host pre-slices and
  pre-transposes inputs to channel-major [64, 32, 128] segments.
- Encoder matmuls: fp32 for the q_s / k_t passes (their embeddings feed the
  cdist/argmin whose top-2 gaps are ~2e-4 — needs full fp32), float32r
  (tf32-rate) for q_t / k_s (their consumers have loose tolerance).
- Tiny [64, 32] embeddings are AllGathered across the 8 cores; every core
  then computes the full-batch tail (MLPs, cdist/argmax, block logits) and
  1/8 of the big queue logits columns.
- Host reassembles the full outputs.
"""
import numpy as np

import concourse.bass as bass
import concourse.tile as tile
from concourse import mybir, bacc
from concourse.bass_utils import run_bass_kernel_spmd

F32 = mybir.dt.float32
F32R = mybir.dt.float32r
I32 = mybir.dt.int32
U32 = mybir.dt.uint32
AF = mybir.ActivationFunctionType
OP = mybir.AluOpType

N_CORES = 8
B = 256
BL = B // N_CORES          # 32 batch rows per core
L = 1024
C = 64
NUM_LAYERS = 5
KQ = 24576
KQL = KQ // N_CORES        # 3072 queue columns per core
HID = 256
NST = 32
W = 125                    # receptive field of the TCN at the last step
SEGW = 128                 # padded per-batch segment width
T_INV = 1.0 / 0.07
EPS = 1e-12

# per-layer needed input ranges: R[l] timesteps of x_l feed the final output
R = [125, 121, 113, 97, 65, 1]
# gather order of the four passes
PASS_QS, PASS_QT, PASS_KS, PASS_KT = 0, 1, 2, 3
# (pass, precision): q_s & k_t exact
PASS_DT = {PASS_QS: F32, PASS_KT: F32, PASS_QT: F32R, PASS_KS: F32R}


def _conv_groups(c):
    """Split 32 segments into groups with group_size*c <= 512."""
    g = min(32, max(1, 512 // c))
    out = []
    s = 0
    while s < 32:
        out.append((s, min(g, 32 - s)))
        s += g
    return out


def build_nc():
    nc = bacc.Bacc("TRN2", target_bir_lowering=False, debug=False,
                   num_devices=N_CORES)

    # ---------------- DRAM parameters ----------------
    seq_in = {}
    for p in range(4):
        seq_in[p] = nc.dram_tensor(f"seq{p}", [C, BL, SEGW], PASS_DT[p],
                                   kind="ExternalInput")
    wpk = {F32: nc.dram_tensor("wpk_f", [30, C, C], F32, kind="ExternalInput"),
           F32R: nc.dram_tensor("wpk_r", [30, C, C], F32R, kind="ExternalInput")}
    bias_d = nc.dram_tensor("biasd", [10, C], F32, kind="ExternalInput")
    pjw1_d = nc.dram_tensor("pjw1", [C, HID], F32, kind="ExternalInput")
    pjw2_d = nc.dram_tensor("pjw2p", [128, 2, C], F32, kind="ExternalInput")
    pjb1_d = nc.dram_tensor("pjb1p", [128, 2], F32, kind="ExternalInput")
    pjb2_d = nc.dram_tensor("pjb2p", [C, 1], F32, kind="ExternalInput")
    prw1_d = nc.dram_tensor("prw1", [C + NST, HID], F32, kind="ExternalInput")
    prw2_d = nc.dram_tensor("prw2p", [128, 2, 2], F32, kind="ExternalInput")
    prb1_d = nc.dram_tensor("prb1p", [128, 2], F32, kind="ExternalInput")
    prb2_d = nc.dram_tensor("prb2p", [2, 1], F32, kind="ExternalInput")
    dw1_d = nc.dram_tensor("dw1", [C, HID], F32, kind="ExternalInput")
    dw2_d = nc.dram_tensor("dw2p", [128, 2, 1], F32, kind="ExternalInput")
    db1_d = nc.dram_tensor("db1p", [128, 2], F32, kind="ExternalInput")
    db2_d = nc.dram_tensor("db2p", [1, 1], F32, kind="ExternalInput")
    statT_d = nc.dram_tensor("statT", [NST, B], F32, kind="ExternalInput")
    qs_d = nc.dram_tensor("queue_s", [C, KQL], F32R, kind="ExternalInput")
    qt_d = nc.dram_tensor("queue_t", [C, KQL], F32R, kind="ExternalInput")

    out_qs = {"s": nc.dram_tensor("oq_s", [B, KQL], F32, kind="ExternalOutput"),
              "t": nc.dram_tensor("oq_t", [B, KQL], F32, kind="ExternalOutput")}
    out_blk = {"s": nc.dram_tensor("ob_s", [B, B], F32, kind="ExternalOutput"),
               "t": nc.dram_tensor("ob_t", [B, B], F32, kind="ExternalOutput")}
    out_ts = nc.dram_tensor("o_ts", [B, B], F32, kind="ExternalOutput")
    out_lab = nc.dram_tensor("o_lab", [B], I32, kind="ExternalOutput")
    out_pd = nc.dram_tensor("o_pd", [2 * B], F32, kind="ExternalOutput")
    out_ys = nc.dram_tensor("o_ys", [2, B], F32, kind="ExternalOutput")

    with tile.TileContext(nc) as tc:
        with (
            tc.tile_pool(name="const", bufs=1) as constp,
            tc.tile_pool(name="act", bufs=1) as actp,
            tc.tile_pool(name="small", bufs=1) as smallp,
            tc.tile_pool(name="work", bufs=2) as workp,
            tc.tile_pool(name="ps", bufs=8, space="PSUM") as psp,
            tc.tile_pool(name="dram", bufs=1, space="DRAM") as dramp,
        ):
            # ---------------- constants / weights ----------------
            w_sb = {}
            for dt in (F32, F32R):
                t = constp.tile([C, 30, C], dt, tag=f"w_{dt}")
                nc.sync.dma_start(out=t[:], in_=wpk[dt][:].rearrange("i p o -> p i o"))
                w_sb[dt] = t
            bias_sb = constp.tile([C, 10], F32, tag="bias")
            nc.sync.dma_start(out=bias_sb[:], in_=bias_d[:].rearrange("i c -> c i"))
            ones64 = constp.tile([C, 1], F32, tag="ones64")
            nc.vector.memset(ones64[:], 1.0)
            ones1x64 = constp.tile([1, C], F32, tag="ones1x64")
            nc.vector.memset(ones1x64[:], 1.0)
            ones1x128 = constp.tile([1, 128], F32, tag="ones1x128")
            nc.vector.memset(ones1x128[:], 1.0)

            pjw1 = constp.tile([C, HID], F32, tag="pjw1")
            nc.sync.dma_start(out=pjw1[:], in_=pjw1_d[:])
            pjw2 = constp.tile([128, 2, C], F32, tag="pjw2")
            nc.sync.dma_start(out=pjw2[:], in_=pjw2_d[:])
            pjb1 = constp.tile([128, 2], F32, tag="pjb1")
            nc.sync.dma_start(out=pjb1[:], in_=pjb1_d[:])
            pjb2 = constp.tile([C, 1], F32, tag="pjb2")
            nc.sync.dma_start(out=pjb2[:], in_=pjb2_d[:])
            prw1 = constp.tile([C + NST, HID], F32, tag="prw1")
            nc.sync.dma_start(out=prw1[:], in_=prw1_d[:])
            prw2 = constp.tile([128, 2, 2], F32, tag="prw2")
            nc.sync.dma_start(out=prw2[:], in_=prw2_d[:])
            prb1 = constp.tile([128, 2], F32, tag="prb1")
            nc.sync.dma_start(out=prb1[:], in_=prb1_d[:])
            prb2 = constp.tile([2, 1], F32, tag="prb2")
            nc.sync.dma_start(out=prb2[:], in_=prb2_d[:])
            dw1 = constp.tile([C, HID], F32, tag="dw1")
            nc.sync.dma_start(out=dw1[:], in_=dw1_d[:])
            dw2 = constp.tile([128, 2, 1], F32, tag="dw2")
            nc.sync.dma_start(out=dw2[:], in_=dw2_d[:])
            db1 = constp.tile([128, 2], F32, tag="db1")
            nc.sync.dma_start(out=db1[:], in_=db1_d[:])
            db2 = constp.tile([1, 1], F32, tag="db2")
            nc.sync.dma_start(out=db2[:], in_=db2_d[:])

            queue_sb = {}
            for tag, d in (("s", qs_d), ("t", qt_d)):
                t = constp.tile([C, KQL], F32R, tag=f"queue_{tag}")
                nc.sync.dma_start(out=t[:], in_=d[:])
                queue_sb[tag] = t

            # ---------------- TCN encoder: 4 passes ----------------
            x5c = {}
            for p in range(4):
                dt = PASS_DT[p]
                x = actp.tile([C, BL, SEGW], dt, tag="x0")
                nc.sync.dma_start(out=x[:], in_=seq_in[p][:])
                for lyr in range(NUM_LAYERS):
                    d = 2 ** lyr
                    c2 = R[lyr + 1] + (R[lyr + 1] % 2)   # fp32r needs even N
                    c1 = c2 + 2 * d
                    wbase = lyr * 6
                    h1 = actp.tile([C, BL, SEGW], dt, tag=f"h1{lyr % 2}")
                    # conv1 + bias + relu
                    o1 = SEGW - c1
                    for (s0, g) in _conv_groups(c1):
                        pm = psp.tile([C, g, c1], F32, tag="pp")
                        pmv = pm[:]
                        for k in range(3):
                            off = o1 - (2 - k) * d
                            nc.tensor.matmul(
                                pmv, w_sb[dt][:, wbase + k, :],
                                x[:, s0:s0 + g, off:off + c1],
                                start=(k == 0), stop=(k == 2))
                        nc.scalar.activation(
                            h1[:, s0:s0 + g, o1:SEGW], pmv, AF.Relu,
                            bias=bias_sb[:, 2 * lyr:2 * lyr + 1])
                    # conv2 + bias + relu, then residual (+relu on layer 0)
                    o2 = SEGW - c2
                    xn = actp.tile([C, BL, SEGW], dt, tag=f"x{(lyr + 1) % 3}")
                    for (s0, g) in _conv_groups(c2):
                        pm = psp.tile([C, g, c2], F32, tag="pp")
                        pmv = pm[:]
                        for k in range(3):
                            off = o2 - (2 - k) * d
                            nc.tensor.matmul(
                                pmv, w_sb[dt][:, wbase + 3 + k, :],
                                h1[:, s0:s0 + g, off:off + c2],
                                start=(k == 0), stop=(k == 2))
                        h2g = workp.tile([C, g, c2], F32, tag="h2g")
                        nc.scalar.activation(
                            h2g[:], pmv, AF.Relu,
                            bias=bias_sb[:, 2 * lyr + 1:2 * lyr + 2])
                        if lyr == 0:
                            tmp = workp.tile([C, g, c2], F32, tag="res0")
                            nc.vector.tensor_tensor(
                                tmp[:], h2g[:], x[:, s0:s0 + g, o2:SEGW], OP.add)
                            nc.vector.tensor_scalar_max(
                                xn[:, s0:s0 + g, o2:SEGW], tmp[:], 0.0)
                        else:
                            nc.vector.tensor_tensor(
                                xn[:, s0:s0 + g, o2:SEGW], h2g[:],
                                x[:, s0:s0 + g, o2:SEGW], OP.add)
                    x = xn
                # copy out the last-step features [64, 32]
                xc = smallp.tile([C, BL], F32, tag=f"x5c_{p}")
                nc.vector.tensor_copy(xc[:], x[:, :, SEGW - 1:SEGW])
                x5c[p] = xc

            # ---------------- normalize the four embeddings ----------------
            nrm_ps = psp.tile([1, 512], F32, tag="pp")
            sq = {}
            for p in range(4):
                s = workp.tile([C, BL], F32, tag="sq")
                nc.vector.tensor_tensor(s[:], x5c[p][:], x5c[p][:], OP.mult)
                nc.tensor.matmul(nrm_ps[0:1, 32 * p:32 * p + 32], ones64[:], s[:],
                                 start=True, stop=True)
                sq[p] = s
            nrm = smallp.tile([1, 128], F32, tag="nrm")
            nc.scalar.sqrt(nrm[:], nrm_ps[0:1, 0:128])
            nc.vector.tensor_scalar_max(nrm[:], nrm[:], EPS)
            rec = smallp.tile([1, 128], F32, tag="rec")
            nc.vector.reciprocal(rec[:], nrm[:])
            bc_ps = psp.tile([C, 128], F32, tag="pp")
            nc.tensor.matmul(bc_ps[:], ones1x64[:], rec[:], start=True, stop=True)
            emb = {}
            for p in range(4):
                e = smallp.tile([C, BL], F32, tag=f"emb_{p}")
                nc.vector.tensor_tensor(e[:], x5c[p][:], bc_ps[:, 32 * p:32 * p + 32],
                                        OP.mult)
                emb[p] = e

            # ---------------- all-gather the embeddings ----------------
            b_in = dramp.tile([4 * C, BL], F32)
            b_out = dramp.tile([N_CORES * 4 * C, BL], F32)
            for p in range(4):
                nc.sync.dma_start(out=b_in[C * p:C * (p + 1), :], in_=emb[p][:])
            nc.gpsimd.collective_compute(
                "AllGather", OP.bypass,
                replica_groups=[list(range(N_CORES))],
                ins=[b_in.opt()], outs=[b_out.opt()])
            emb_all = smallp.tile([C, 4, B], F32, tag="emb_all")
            src = b_out[:].rearrange("(k p c) b -> p c k b", k=N_CORES, p=4)
            for p in range(4):
                nc.sync.dma_start(
                    out=emb_all[:, p, :].rearrange("c (k b) -> c k b", k=N_CORES),
                    in_=src[p])

            q_s_all = emb_all[:, PASS_QS, :]
            q_t_all = emb_all[:, PASS_QT, :]
            k_s_all = emb_all[:, PASS_KS, :]
            k_t_all = emb_all[:, PASS_KT, :]

            # ---------------- projection MLP p_q = l2n(mlp(q)) * (1/T) -------
            pq_scaled = {}
            pqn_ps = psp.tile([1, 512], F32, tag="pp")
            praw = {}
            for i, (tag, qv) in enumerate((("s", q_s_all), ("t", q_t_all))):
                hts = []
                for m in range(2):
                    hp = psp.tile([128, 256], F32, tag="pp")
                    nc.tensor.matmul(hp[:], pjw1[:, 128 * m:128 * (m + 1)], qv,
                                     start=True, stop=True)
                    ht = workp.tile([128, B], F32, tag=f"pqh{m}")
                    nc.scalar.activation(ht[:], hp[:], AF.Relu,
                                         bias=pjb1[:, m:m + 1])
                    hts.append(ht)
                op = psp.tile([C, 256], F32, tag="pp")
                for s in range(2):
                    nc.tensor.matmul(op[:], pjw2[:, s, :], hts[s][:],
                                     start=(s == 0), stop=(s == 1))
                pr = smallp.tile([C, B], F32, tag=f"praw_{tag}")
                nc.scalar.activation(pr[:], op[:], AF.Identity, bias=pjb2[:])
                praw[tag] = pr
                s2 = workp.tile([C, B], F32, tag="sq2")
                nc.vector.tensor_tensor(s2[:], pr[:], pr[:], OP.mult)
                nc.tensor.matmul(pqn_ps[0:1, 256 * i:256 * (i + 1)], ones64[:],
                                 s2[:], start=True, stop=True)
            pqn = smallp.tile([1, 512], F32, tag="pqn")
            nc.scalar.sqrt(pqn[:], pqn_ps[:])
            nc.vector.tensor_scalar_max(pqn[:], pqn[:], EPS)
            pqr = smallp.tile([1, 512], F32, tag="pqr")
            nc.vector.reciprocal(pqr[:], pqn[:])
            nc.vector.tensor_scalar_mul(pqr[:], pqr[:], T_INV)
            for i, tag in enumerate(("s", "t")):
                bp = psp.tile([C, 512], F32, tag="pp")
                nc.tensor.matmul(bp[0:C, 0:256], ones1x64[:],
                                 pqr[0:1, 256 * i:256 * (i + 1)],
                                 start=True, stop=True)
                pq = smallp.tile([C, B], F32, tag=f"pq_{tag}")
                nc.vector.tensor_tensor(pq[:], praw[tag][:], bp[0:C, 0:256],
                                        OP.mult)
                pq_scaled[tag] = pq

            # ---------------- queue logits (f32r, col-sharded) ----------------
            for tag in ("s", "t"):
                pq_r = smallp.tile([C, B], F32R, tag=f"pqr_{tag}")
                nc.vector.tensor_copy(pq_r[:], pq_scaled[tag][:])
                for m in range(2):
                    for n in range(KQL // 512):
                        qp = psp.tile([128, 512], F32, tag="pp")
                        nc.tensor.matmul(
                            qp[:], pq_r[:, 128 * m:128 * (m + 1)],
                            queue_sb[tag][:, 512 * n:512 * (n + 1)],
                            start=True, stop=True)
                        qs = workp.tile([128, 512], F32, tag="qout")
                        nc.vector.tensor_copy(qs[:], qp[:])
                        nc.sync.dma_start(
                            out=out_qs[tag][128 * m:128 * (m + 1),
                                            512 * n:512 * (n + 1)],
                            in_=qs[:])

            # ---------------- block logits + logits_ts (fp32) ----------------
            qt_scaled = smallp.tile([C, B], F32, tag="qt_scaled")
            nc.scalar.mul(qt_scaled[:], q_t_all, T_INV)
            blocks = [("s", pq_scaled["s"][:], k_s_all, out_blk["s"]),
                      ("t", pq_scaled["t"][:], k_t_all, out_blk["t"]),
                      ("ts", qt_scaled[:], q_s_all, out_ts)]
            for tag, lhs, rhs, od in blocks:
                for m in range(2):
                    bp2 = psp.tile([128, 256], F32, tag="pp")
                    nc.tensor.matmul(bp2[:], lhs[:, 128 * m:128 * (m + 1)], rhs,
                                     start=True, stop=True)
                    bs = workp.tile([128, B], F32, tag="bout")
                    nc.vector.tensor_copy(bs[:], bp2[:])
                    nc.sync.dma_start(out=od[128 * m:128 * (m + 1), :], in_=bs[:])

            # ---------------- cdist + argmin (exact fp32) ----------------
            sqq = workp.tile([C, B], F32, tag="sqq")
            nc.vector.tensor_tensor(sqq[:], q_s_all, q_s_all, OP.mult)
            nq_ps = psp.tile([1, 256], F32, tag="pp")
            nc.tensor.matmul(nq_ps[:], ones64[:], sqq[:], start=True, stop=True)
            nqs = smallp.tile([1, B], F32, tag="nqs")
            nc.scalar.copy(nqs[:], nq_ps[:])
            b2_ps = psp.tile([128, 256], F32, tag="pp")
            nc.tensor.matmul(b2_ps[:], ones1x128[:], nqs[:], start=True, stop=True)
            b2_sb = workp.tile([128, B], F32, tag="b2_sb")
            nc.vector.tensor_copy(b2_sb[:], b2_ps[:])
            for m in range(2):
                s_ps = psp.tile([128, 256], F32, tag="pp")
                nc.tensor.matmul(s_ps[:], k_t_all[:, 128 * m:128 * (m + 1)],
                                 q_s_all, start=True, stop=True)
                a_sb = workp.tile([128, B], F32, tag="a_sb")
                nc.vector.scalar_tensor_tensor(a_sb[:], s_ps[:], 2.0, b2_sb[:],
                                               OP.mult, OP.subtract)
                m8 = workp.tile([128, 8], F32, tag="m8")
                nc.vector.max(m8[:], a_sb[:])
                i8 = workp.tile([128, 8], U32, tag="i8")
                nc.vector.max_index(i8[:], m8[:], a_sb[:])
                li = workp.tile([128, 1], I32, tag="li")
                nc.vector.tensor_copy(li[:], i8[:, 0:1])
                nc.sync.dma_start(out=out_lab[128 * m:128 * (m + 1)], in_=li[:])

            # ---------------- pred_domain (fp32) ----------------
            q_rev = emb_all[:, 0:2, :]   # [64, 2, 256] == [q_s | q_t]
            hds = []
            for m in range(2):
                hp = psp.tile([128, 512], F32, tag="pp")
                nc.tensor.matmul(hp[:], dw1[:, 128 * m:128 * (m + 1)], q_rev,
                                 start=True, stop=True)
                hd = workp.tile([128, 2 * B], F32, tag=f"hd{m}")
                nc.scalar.activation(hd[:], hp[:], AF.Relu, bias=db1[:, m:m + 1])
                hds.append(hd)
            pd_ps = psp.tile([1, 512], F32, tag="pp")
            for s in range(2):
                nc.tensor.matmul(pd_ps[:], dw2[:, s, :], hds[s][:],
                                 start=(s == 0), stop=(s == 1))
            pd_sb = workp.tile([1, 2 * B], F32, tag="pd_sb")
            nc.scalar.activation(pd_sb[:], pd_ps[:], AF.Identity, bias=db2[:])
            nc.sync.dma_start(out=out_pd[:], in_=pd_sb[:])

            # ---------------- y_s head (fp32) ----------------
            rhs96 = workp.tile([C + NST, B], F32, tag="rhs96")
            nc.vector.tensor_copy(rhs96[0:C, :], q_s_all)
            nc.sync.dma_start(out=rhs96[C:C + NST, :], in_=statT_d[:])
            hys = []
            for m in range(2):
                hp = psp.tile([128, 256], F32, tag="pp")
                nc.tensor.matmul(hp[:], prw1[:, 128 * m:128 * (m + 1)], rhs96[:],
                                 start=True, stop=True)
                hy = workp.tile([128, B], F32, tag=f"hy{m}")
                nc.scalar.activation(hy[:], hp[:], AF.Relu, bias=prb1[:, m:m + 1])
                hys.append(hy)
            ys_ps = psp.tile([2, 256], F32, tag="pp")
            for s in range(2):
                nc.tensor.matmul(ys_ps[:], prw2[:, s, :], hys[s][:],
                                 start=(s == 0), stop=(s == 1))
            ys_sb = workp.tile([2, B], F32, tag="ys_sb")
            nc.scalar.activation(ys_sb[:], ys_ps[:], AF.Identity, bias=prb2[:])
            nc.sync.dma_start(out=out_ys[:], in_=ys_sb[:])

    nc.compile()
    return nc


_NC = None


def _get_nc():
    global _NC
    if _NC is None:
        _NC = build_nc()
    return _NC


def make_in_maps(inputs):
    """Host-side prepacking: slice/transpose/shard the full inputs."""
    f = np.float32
    seq_names = {PASS_QS: "sequence_q_s", PASS_QT: "sequence_q_t",
                 PASS_KS: "sequence_k_s", PASS_KT: "sequence_k_t"}
    c1w, c1b = np.asarray(inputs["c1w"], f), np.asarray(inputs["c1b"], f)
    c2w, c2b = np.asarray(inputs["c2w"], f), np.asarray(inputs["c2b"], f)
    wpk = np.zeros((30, C, C), f)
    biasd = np.zeros((10, C), f)
    for lyr in range(NUM_LAYERS):
        for k in range(3):
            wpk[lyr * 6 + k] = c1w[lyr, :, :, k].T
            wpk[lyr * 6 + 3 + k] = c2w[lyr, :, :, k].T
        biasd[2 * lyr] = c1b[lyr]
        biasd[2 * lyr + 1] = c2b[lyr]

    pjw2p = np.asarray(inputs["pjw2"], f).reshape(2, 128, C).transpose(1, 0, 2)
    pjb1p = np.asarray(inputs["pjb1"], f).reshape(2, 128).T
    prw2p = np.asarray(inputs["prw2"], f).reshape(2, 128, 2).transpose(1, 0, 2)
    prb1p = np.asarray(inputs["prb1"], f).reshape(2, 128).T
    dw2p = np.asarray(inputs["dw2"], f).reshape(2, 128, 1).transpose(1, 0, 2)
    db1p = np.asarray(inputs["db1"], f).reshape(2, 128).T

    shared = {
        "wpk_f": np.ascontiguousarray(wpk),
        "wpk_r": np.ascontiguousarray(wpk),
        "biasd": biasd,
        "pjw1": np.ascontiguousarray(np.asarray(inputs["pjw1"], f)),
        "pjw2p": np.ascontiguousarray(pjw2p),
        "pjb1p": np.ascontiguousarray(pjb1p),
        "pjb2p": np.asarray(inputs["pjb2"], f).reshape(C, 1),
        "prw1": np.ascontiguousarray(np.asarray(inputs["prw1"], f)),
        "prw2p": np.ascontiguousarray(prw2p),
        "prb1p": np.ascontiguousarray(prb1p),
        "prb2p": np.asarray(inputs["prb2"], f).reshape(2, 1),
        "dw1": np.ascontiguousarray(np.asarray(inputs["dw1"], f)),
        "dw2p": np.ascontiguousarray(dw2p),
        "db1p": np.ascontiguousarray(db1p),
        "db2p": np.asarray(inputs["db2"], f).reshape(1, 1),
        "statT": np.ascontiguousarray(np.asarray(inputs["static_s"], f).T),
    }
    in_maps = []
    for c in range(N_CORES):
        m = dict(shared)
        for p in range(4):
            seq = np.asarray(inputs[seq_names[p]], f)
            arr = np.zeros((C, BL, SEGW), f)
            arr[:, :, SEGW - W:] = seq[c * BL:(c + 1) * BL, L - W:, :] \
                .transpose(2, 0, 1)
            m[f"seq{p}"] = arr
        m["queue_s"] = np.ascontiguousarray(
            np.asarray(inputs["queue_s"], f)[:, c * KQL:(c + 1) * KQL])
        m["queue_t"] = np.ascontiguousarray(
            np.asarray(inputs["queue_t"], f)[:, c * KQL:(c + 1) * KQL])
        in_maps.append(m)
    return in_maps


def assemble(results):
    """Host-side reassembly of full outputs from per-core results."""
    f = np.float32
    r0 = results[0]
    logits_s = np.concatenate([r0["ob_s"]] + [results[c]["oq_s"]
                                              for c in range(N_CORES)], axis=1)
    logits_t = np.concatenate([r0["ob_t"]] + [results[c]["oq_t"]
                                              for c in range(N_CORES)], axis=1)
    logits_ts = r0["o_ts"]
    labels_ts = r0["o_lab"].astype(np.int32)
    pred_domain = r0["o_pd"].reshape(2 * B, 1)
    y_s = np.ascontiguousarray(r0["o_ys"].T)
    labels_s = np.arange(B, dtype=np.int32)
    labels_t = np.arange(B, dtype=np.int32)
    labels_domain = np.concatenate([np.ones((B, 1), f), np.zeros((B, 1), f)], 0)
    return (logits_s, labels_s, logits_t, labels_t, logits_ts, labels_ts,
            pred_domain, labels_domain, y_s)


def kernel(**inputs):
    nc = _get_nc()
    in_maps = make_in_maps(inputs)
    last_err = None
    for _ in range(3):
        try:
            res = run_bass_kernel_spmd(nc, in_maps, list(range(N_CORES)))
            return assemble(res.results)
        except Exception as e:  # compile/exec flakes: retry
            last_err = e
    raise last_err
